# revision 9
# baseline (speedup 1.0000x reference)
"""Multi-head cross-attention kernel for 8 Trainium2 NeuronCores.

Sharding: core = (batch, head-group) — cores 0-3 take batch 0, cores 4-7
batch 1; core m%4 takes heads [4*(m%4), 4*(m%4)+4). Each core projects
q/k/v for its 4 heads, runs fused (no-max) softmax attention fully
on-chip, and produces a partial out-projection (transposed). The host
sums the four per-batch partials and transposes back.

Shapes (hardcoded per problem spec):
  query_states [2, 2048, 1024], key/value_states [2, 4096, 1024],
  Wq/Wk/Wv/Wo [1024, 1024] (torch Linear layout, applied as x @ W.T).

On-chip layouts per core (b = batch, hg = head group, s = 256-dim slice):
  xq = query[b].T [1024, 2048], xk/xv = key/value[b].T [1024, 4096]
  wqT/wkT/wvT = W[s,:] .T -> [1024, 256], woT = Wo[:, s].T -> [256, 1024]
  QT [256, 2048] = (q @ Wq_s.T).T ; KT [256, 4096] ; V [4096, 4x65 (ones col)]
  scores.T tiles [128 kv, 1024 q] -> exp (scale folded) -> X += V'.T @ expS
  X row 64 = softmax denominator (ones column of V'); normalize via
  reciprocal + gpsimd partition_broadcast; out.T [1024, 2048] = WoT.T @ X.
"""

import os

import numpy as np

import concourse.bass as bass
import concourse.tile as tile
from concourse import bacc, mybir
from concourse.bass_utils import run_bass_kernel_spmd

B, QL, KVL, HIDDEN = 2, 2048, 4096, 1024
N_HEADS, HEAD_DIM = 16, 64
SCALE = HEAD_DIM**-0.5
N_CORES = 8
HPC = 4  # heads per core
DS = HPC * HEAD_DIM  # 256: per-core hidden slice

F32 = mybir.dt.float32
F32R = mybir.dt.float32r
# Matmul operand dtype: float32r runs the PE at full rate (fp32 is 4x
# slower); ~1e-4 scale-relative rounding. Set BASS_MM_F32=1 for full fp32.
MM_DT = F32 if os.environ.get("BASS_MM_F32") else F32R


def _build_program(debug=False):
    nc = bacc.Bacc(None)
    xq = nc.dram_tensor("xq", [HIDDEN, QL], MM_DT, kind="ExternalInput")
    xk = nc.dram_tensor("xk", [HIDDEN, KVL], MM_DT, kind="ExternalInput")
    xv = nc.dram_tensor("xv", [HIDDEN, KVL], MM_DT, kind="ExternalInput")
    wqT = nc.dram_tensor("wqT", [HIDDEN, DS], MM_DT, kind="ExternalInput")
    wkT = nc.dram_tensor("wkT", [HIDDEN, DS], MM_DT, kind="ExternalInput")
    wvT = nc.dram_tensor("wvT", [HIDDEN, DS], MM_DT, kind="ExternalInput")
    woT = nc.dram_tensor("woT", [DS, HIDDEN], MM_DT, kind="ExternalInput")
    outT = nc.dram_tensor("outT", [HIDDEN, QL], F32, kind="ExternalOutput")

    HC = HIDDEN // 128  # 8 contraction chunks over hidden
    DC = DS // 128  # 2 chunks over the per-core 256-dim slice
    KVC = KVL // 128  # 32 kv chunks

    with tile.TileContext(nc) as tc:
        with (
            tc.tile_pool(name="persist", bufs=1) as persist,
            tc.tile_pool(name="wpool", bufs=1) as wpool,
        ):
            # Long-lived SBUF tensors.
            KT = persist.tile([128, DC, KVL], MM_DT)  # K.T for this head group
            # Per-head Q with the pair-partner's 64 rows zeroed: score matmuls
            # contract over all 128 partitions (zero rows annihilate the other
            # head) so the PE looks fully busy to HAM and stays at 2.4 GHz.
            QT = persist.tile([128, HPC, QL], MM_DT)
            Vsb = persist.tile([128, KVC, HPC, HEAD_DIM + 1], MM_DT)
            Xsb = persist.tile([128, DC, QL], MM_DT)  # normalized attn out (.T)
            wo_sb = persist.tile([128, DC, HIDDEN], MM_DT)
            ones_sb = persist.tile([128, HEAD_DIM], F32)
            nc.vector.memset(QT.bitcast(F32), 0.0)
            nc.vector.memset(ones_sb, 1.0)

            wq_sb = wpool.tile([128, HC, DS], MM_DT, tag="wq")
            wk_sb = wpool.tile([128, HC, DS], MM_DT, tag="wk")
            wv_sb = wpool.tile([128, HC, DS], MM_DT, tag="wv")
            nc.sync.dma_start(wq_sb[:], wqT.rearrange("(c p) m -> p c m", p=128))
            nc.sync.dma_start(wk_sb[:], wkT.rearrange("(c p) m -> p c m", p=128))
            nc.sync.dma_start(wv_sb[:], wvT.rearrange("(c p) m -> p c m", p=128))
            nc.sync.dma_start(wo_sb[:], woT.rearrange("(c p) m -> p c m", p=128))

            # ones column of V' (softmax denominator accumulator); bitcast:
            # ISA has no float32r memset, and fp32 bits of 1.0 are identical
            nc.vector.memset(Vsb[:, :, :, HEAD_DIM : HEAD_DIM + 1].bitcast(F32), 1.0)

            # ---- K projection: KT[dk, kv] = sum_h wkT[h, dk] * xk[h, kv] ----
            with (
                tc.tile_pool(name="xstream", bufs=3) as xs,
                tc.tile_pool(name="pproj", bufs=8, space="PSUM") as pp,
            ):
                for half in range(2):
                    ps = [
                        [pp.tile([128, 512], F32, tag="psk", name=f"psk_{dk}_{t}") for t in range(4)]
                        for dk in range(DC)
                    ]
                    for h in range(HC):
                        xt = xs.tile([128, 2048], MM_DT, tag="xk")
                        nc.sync.dma_start(
                            xt[:], xk[h * 128 : (h + 1) * 128, half * 2048 : half * 2048 + 2048]
                        )
                        for dk in range(DC):
                            for t in range(4):
                                nc.tensor.matmul(
                                    ps[dk][t][:],
                                    wk_sb[:, h, dk * 128 : (dk + 1) * 128],
                                    xt[:, t * 512 : (t + 1) * 512],
                                    start=(h == 0),
                                    stop=(h == HC - 1),
                                )
                    for dk in range(DC):
                        for t in range(4):
                            nc.vector.tensor_copy(
                                KT[:, dk, half * 2048 + t * 512 : half * 2048 + (t + 1) * 512],
                                ps[dk][t][:],
                            )

                    # ---- V projection: V[kv, dv] = sum_h xv[h, kv] * wvT[h, dv] ----
                    # (kv on partitions so V can be the attn@V stationary operand)
                    for grp in range(half * 2, half * 2 + 2):  # 4 groups of 8 kv chunks
                        psv = [pp.tile([128, 512], F32, tag="psk", name=f"psv_{c}")[:, :DS] for c in range(8)]
                        for h in range(HC):
                            xvt = xs.tile([128, 1024], MM_DT, tag="xv")
                            nc.sync.dma_start(
                                xvt[:],
                                xv[h * 128 : (h + 1) * 128, grp * 1024 : (grp + 1) * 1024],
                            )
                            for c in range(8):
                                nc.tensor.matmul(
                                    psv[c][:],
                                    xvt[:, c * 128 : (c + 1) * 128],
                                    wv_sb[:, h, :],
                                    start=(h == 0),
                                    stop=(h == HC - 1),
                                )
                        for c in range(8):
                            nc.vector.tensor_copy(
                                Vsb[:, grp * 8 + c, :, 0:HEAD_DIM],
                                psv[c].rearrange("p (hh d) -> p hh d", hh=HPC),
                            )

                # ---- Q projection ----
                psq = [
                    [pp.tile([128, 512], F32, tag="psk", name=f"psq_{dq}_{t}") for t in range(4)]
                    for dq in range(DC)
                ]
                for h in range(HC):
                    xqt = xs.tile([128, 2048], MM_DT, tag="xk")
                    nc.sync.dma_start(xqt[:], xq[h * 128 : (h + 1) * 128, :])
                    for dq in range(DC):
                        for t in range(4):
                            nc.tensor.matmul(
                                psq[dq][t][:],
                                wq_sb[:, h, dq * 128 : (dq + 1) * 128],
                                xqt[:, t * 512 : (t + 1) * 512],
                                start=(h == 0),
                                stop=(h == HC - 1),
                            )
                for h in range(HPC):
                    pb = (h % 2) * 64
                    for t in range(4):
                        nc.vector.tensor_copy(
                            QT[pb : pb + 64, h, t * 512 : (t + 1) * 512],
                            psq[h // 2][t][pb : pb + 64, :],
                        )

            # ---- Attention (fused, no-max softmax) ----
            with (
                tc.tile_pool(name="attn_sb", bufs=3) as asb,
                tc.tile_pool(name="norm_sb", bufs=2) as nsb,
                tc.tile_pool(name="pstg", bufs=2, space="PSUM") as pstg,
                tc.tile_pool(name="px", bufs=4, space="PSUM") as px,
            ):
                for h in range(HPC):
                    pb = (h % 2) * 64  # partition base of this head's 64 dims
                    hc = h // 2
                    for qh in range(2):  # q halves of 1024
                        q0 = qh * 1024
                        X = [px.tile([65, 512], F32, tag="x", name=f"x_{t}") for t in range(2)]
                        for c in range(KVC):
                            stg = pstg.tile([128, 1024], F32, tag="stg")
                            for t in range(2):
                                nc.tensor.matmul(
                                    stg[:, t * 512 : (t + 1) * 512],
                                    KT[:, hc, c * 128 : (c + 1) * 128],
                                    QT[:, h, q0 + t * 512 : q0 + (t + 1) * 512],
                                    start=True,
                                    stop=True,
                                )
                            eS = asb.tile([128, 1024], MM_DT, tag="expS")
                            nc.scalar.activation(
                                out=eS[:],
                                in_=stg[:],
                                func=mybir.ActivationFunctionType.Exp,
                                scale=SCALE,
                            )
                            for t in range(2):
                                nc.tensor.matmul(
                                    X[t][:],
                                    Vsb[:, c, h, :],
                                    eS[:, t * 512 : (t + 1) * 512],
                                    start=(c == 0),
                                    stop=(c == KVC - 1),
                                )
                        for t in range(2):
                            # normalize: X[0:64] / X[64] (ones-column denom).
                            # Broadcast 1/denom across partitions via a K=1
                            # matmul (gpsimd partition_broadcast only reads
                            # partition 0; DVE can't cross partitions).
                            recip = nsb.tile([128, 512], F32, tag="recip")
                            nc.vector.reciprocal(recip[64:65, :], X[t][64:65, :])
                            bc_ps = px.tile([65, 512], F32, tag="x", name="bc_ps")
                            nc.tensor.matmul(
                                bc_ps[0:64, :],
                                ones_sb[64:65, :],
                                recip[64:65, :],
                                start=True,
                                stop=True,
                            )
                            bc_sb = nsb.tile([128, 512], F32, tag="bcast")
                            nc.vector.tensor_copy(bc_sb[0:64, :], bc_ps[0:64, :])
                            sc = nsb.tile([128, 512], MM_DT, tag="scratch")
                            nc.vector.tensor_tensor(
                                sc[0:64, :],
                                X[t][0:64, :],
                                bc_sb[0:64, :],
                                mybir.AluOpType.mult,
                            )
                            # partition move 0..64 -> pb..pb+64 via DMA
                            nc.sync.dma_start(
                                Xsb[pb : pb + 64, hc, q0 + t * 512 : q0 + (t + 1) * 512],
                                sc[0:64, :],
                            )

            # ---- Output projection (partial): outT = woT.T @ Xsb ----
            with (
                tc.tile_pool(name="out_sb", bufs=2) as osb,
                tc.tile_pool(name="pso", bufs=4, space="PSUM") as pso,
            ):
                for oc in range(HIDDEN // 128):
                    ot = osb.tile([128, QL], F32, tag="outsb")
                    for t in range(4):
                        p = pso.tile([128, 512], F32, tag="pso")
                        for dv in range(DC):
                            nc.tensor.matmul(
                                p[:],
                                wo_sb[:, dv, oc * 128 : (oc + 1) * 128],
                                Xsb[:, dv, t * 512 : (t + 1) * 512],
                                start=(dv == 0),
                                stop=(dv == DC - 1),
                            )
                        nc.vector.tensor_copy(ot[:, t * 512 : (t + 1) * 512], p[:])
                    nc.sync.dma_start(outT[oc * 128 : (oc + 1) * 128, :], ot[:])

            if debug:
                for name, t in [("KT", KT), ("QT", QT), ("Vsb", Vsb), ("Xsb", Xsb)]:
                    dbg = nc.dram_tensor(
                        f"dbg_{name}", list(t.shape), MM_DT, kind="ExternalOutput"
                    )
                    nc.sync.dma_start(dbg[:], t[:])

    nc.finalize()
    return nc


_PROGRAM = None


def _program():
    global _PROGRAM
    if _PROGRAM is None:
        _PROGRAM = _build_program()
    return _PROGRAM


def _shard_inputs(query_states, key_states, value_states, Wq, Wk, Wv, Wo):
    in_maps = []
    for core in range(N_CORES):
        b = core // HPC
        hg = core % HPC
        s = slice(hg * DS, (hg + 1) * DS)
        in_maps.append(
            {
                "xq": np.ascontiguousarray(query_states[b].T),
                "xk": np.ascontiguousarray(key_states[b].T),
                "xv": np.ascontiguousarray(value_states[b].T),
                "wqT": np.ascontiguousarray(Wq[s, :].T),
                "wkT": np.ascontiguousarray(Wk[s, :].T),
                "wvT": np.ascontiguousarray(Wv[s, :].T),
                "woT": np.ascontiguousarray(Wo[:, s].T),
            }
        )
    return in_maps


def _gather_output(results):
    out = np.empty((B, QL, HIDDEN), np.float32)
    for b in range(B):
        acc = results[b * HPC]["outT"].astype(np.float32)
        for i in range(1, HPC):
            acc = acc + results[b * HPC + i]["outT"]
        out[b] = acc.T
    return out


def run_sharded(inputs, trace=False, tmpdir=None):
    """Run the SPMD kernel; returns (full_output, BassKernelResults)."""
    arrs = {k: np.asarray(v, dtype=np.float32) for k, v in inputs.items()}
    in_maps = _shard_inputs(
        arrs["query_states"],
        arrs["key_states"],
        arrs["value_states"],
        arrs["Wq"],
        arrs["Wk"],
        arrs["Wv"],
        arrs["Wo"],
    )
    res = run_bass_kernel_spmd(
        _program(), in_maps, list(range(N_CORES)), trace=trace, tmpdir=tmpdir
    )
    return _gather_output(res.results), res


def kernel(**inputs):
    out, _ = run_sharded(inputs)
    return out
